# revision 39
# baseline (speedup 1.0000x reference)
"""Trainium2 Bass kernel for MemoryEfficientCrossAttention (v3).

Problem (hardcoded): B=2, Q=2048, K=4096, HIDDEN=1024, HEADS=16, HEAD_DIM=64.
  out = softmax((x_q W_q)(x_k W_k)^T / sqrt(64)) (x_v W_v) W_o

Sharding over 8 NeuronCores: core = b*4 + t
  b in {0,1}: batch;  t in {0..3}: head-quad (4 heads = 256 cols of W_q/k/v).
No duplicated FLOPs: each core projects q/k/v for its quad only, runs
attention for (full Q x its 4 heads), computes the partial out projection
ctx_t @ W_o[256t:256t+256, :]; the host sums the 4 partials per batch.

v3 structure (vs v2):
- Score PSUM tiles [128 k, 2 j, 512 q] per head-pair (2 banks, one full
  bank per head, start&stop per matmul — a start=False matmul must not
  change the PE row offset mid-group on real HW).  One Act exp
  instruction per tile (1024 free elems) -> A tile in SBUF bf16.
- 3-deep score-tile rotation (6 PSUM banks) gives the PE a 1.5-kblock
  runway over the Act engine, hiding cross-engine semaphore latency.
- The Act engine runs ONLY exp (single activation table load); every
  PSUM->SBUF copy, flush, and scale sits on the DVE.
- ctx^T for the out-projection comes from PE is_transpose matmuls (fp32,
  via an identity stationary) instead of the SP-heavy DMA transpose.
- K/V projections are spread as small closures across qb0's k-loop
  (just-in-time, 2-kblock DMA lead); the out-projection of qb is spread
  across qb+1's k-loop in per-qq pieces.
- PSUM: 3x2 banks score rotation + a shared 2-slot 1-bank "aux" ring
  (projection strips, PV scratch, transposes, out-proj accumulators).
"""

import os
import sys
import time

import numpy as np

sys.path.insert(0, "/opt/trn_rl_repo")

from contextlib import ExitStack  # noqa: E402

import concourse.bass as bass  # noqa: E402
import concourse.mybir as mybir  # noqa: E402
import concourse.tile as tile  # noqa: E402
from concourse import bacc  # noqa: E402

F32 = mybir.dt.float32
BF16 = mybir.dt.bfloat16

HID = 1024
HEADS = 16
HD = 64
B = 2
Q = 2048
KL = 4096
NCORE = 8
TC = 256            # head-quad cols per core (4 heads)
NCH = HID // 128    # 8 hidden chunks
NKB = KL // 128     # 32 k-blocks
NQB = Q // 512      # 4 q-blocks of 512
HALF = 256          # queries per score tile
GRP = 8            # k-blocks per PV scratch group
SCALE = HD ** -0.5

EXP = mybir.ActivationFunctionType.Exp
I16 = mybir.dt.int16

# Every XDVE_MOD-th exp tile goes to the DVE as a one-pass Schraudolph
# bit-trick (i16 = s*K + B, bits read as bf16 ~= exp(s), ~2% rms per-weight
# error on those tiles only); 0 disables.  Offloads the Act engine, but the
# in-order DVE queue then delays the score-tile WAR release (3-slot ring),
# which control-adjusted A/B runs suggest costs more than the Act relief
# buys — default off.
XDVE_MOD = int(os.environ.get("XDVE_MOD", "0"))
FEXP_K = 184.6649652337873      # 128 / ln 2
FEXP_B = 16250.5                # 127*128 + C_opt (C = -5.5)

_CACHED_NC = None


def _build():
    nc = bacc.Bacc("TRN2", target_bir_lowering=False, debug=False,
                   num_devices=NCORE)

    # All tensors arrive pre-arranged on the host so every DMA is contiguous
    # per partition (slab-major activations, partition-major weights).
    xqT = nc.dram_tensor("xqT", [4, 128, NCH, 512], BF16,
                         kind="ExternalInput")
    xkT = nc.dram_tensor("xkT", [8, 128, NCH, 512], BF16,
                         kind="ExternalInput")
    xvT = nc.dram_tensor("xvT", [8, 128, NCH, 512], BF16,
                         kind="ExternalInput")
    wq = nc.dram_tensor("wq", [128, NCH, TC], BF16, kind="ExternalInput")
    wk = nc.dram_tensor("wk", [128, NCH, TC], BF16, kind="ExternalInput")
    wv = nc.dram_tensor("wv", [128, NCH, TC], BF16, kind="ExternalInput")
    wo = nc.dram_tensor("wo", [128, 2, HID], BF16, kind="ExternalInput")
    eye = nc.dram_tensor("eye", [128, 128], BF16, kind="ExternalInput")
    # o_part[qb, p, qq, :] = out_partial[qb*512 + qq*128 + p, :]
    o_part = nc.dram_tensor("o_part", [NQB, 128, 4, HID], BF16,
                            kind="ExternalOutput")

    with tile.TileContext(nc, pool_alloc_mode="queue") as tc:
        _pools = ExitStack()
        pp = _pools.enter_context(tc.tile_pool(name="persist", bufs=1))
        wq_sb = pp.tile([128, NCH, TC], BF16)
        wk_sb = pp.tile([128, NCH, TC], BF16)
        wv_sb = pp.tile([128, NCH, TC], BF16)
        wo_sb = pp.tile([128, 2, HID], BF16)
        eye_sb = pp.tile([128, 128], BF16)
        qT = pp.tile([128, 2, Q], BF16)
        kT = pp.tile([128, 2, KL], BF16)
        v_aug = pp.tile([128, NKB, 4, HD + 1], BF16)
        # ctxacc[q, qq16, h, e]: fp32 ctx^T accumulator (e==64 is r)
        ctxacc = pp.tile([128, 16, 4, HD + 1], F32)

        xs = _pools.enter_context(tc.tile_pool(name="xstage", bufs=4))
        # 40 bufs: A(qb,kb16) is sweep-alloc 32 and its last reader (the qq3
        # PV pop of the previous sweep's second group) runs at kb3 of the
        # next sweep; ring reuse distance must exceed 38 allocs or the exp
        # at each sweep boundary stalls on the WAR.
        apool = _pools.enter_context(tc.tile_pool(name="apool", bufs=40))
        mp = _pools.enter_context(tc.tile_pool(name="misc", bufs=2))
        ps = _pools.enter_context(
            tc.tile_pool(name="ps", bufs=2, space="PSUM"))

        nc.vector.memset(v_aug[:, :, :, HD], 1.0)

        def load_xslab(src, s, split=False):
            """[128, NCH, 512] bf16 slab of pre-transposed activations.
            split=True issues two half-slab DMAs so consumers of the first
            hidden-chunk half can start ~1.5us earlier (prologue only)."""
            xsl = xs.tile([128, NCH, 512], BF16, tag="xsl", name="xsl")
            if split:
                h = NCH // 2
                nc.sync.dma_start(xsl[:, 0:h, :], src[s, :, 0:h, :])
                nc.sync.dma_start(xsl[:, h:, :], src[s, :, h:, :])
            else:
                nc.sync.dma_start(xsl[:], src[s])
            return xsl

        def proj_strip(xsl, w_sb, dst, dsl, i):
            """One 512-col strip of a q/k projection into dst[:, i, dsl]."""
            pj = ps.tile([128, 512], F32, tag="aux", name="pjt")
            for hc in range(NCH):
                nc.tensor.matmul(
                    pj[:], w_sb[:, hc, i * 128:(i + 1) * 128],
                    xsl[:, hc, :],
                    start=(hc == 0), stop=(hc == NCH - 1))
            nc.vector.tensor_copy(dst[:, i, dsl], pj[:])

        def v_group(xsl, s, r):
            """One 128-row v-projection group -> v_aug[:, 4s+r]."""
            pj = ps.tile([128, TC], F32, tag="aux", name="pjv",
                         padded_shape=[128, 512])
            for hc in range(NCH):
                nc.tensor.matmul(
                    pj[:], xsl[:, hc, r * 128:(r + 1) * 128],
                    wv_sb[:, hc, :],
                    start=(hc == 0), stop=(hc == NCH - 1))
            nc.vector.tensor_copy(
                v_aug[:, s * 4 + r, :, 0:HD],
                pj[:].rearrange("p (h d) -> p h d", h=4))

        pending_pv = []
        a_tiles = {}

        def emit_scores(qb, kb, pair):
            """Scores for one head-pair: st[:, j, :] = S^T of head 2*pair+j.
            Each head's matmul owns a full PSUM bank (start & stop) so the
            PE row offset (j*64) never changes inside an accumulation
            group — a start=False row switch is illegal on real HW."""
            st = ps.tile([128, 2, 512], F32, tag="st", bufs=3,
                         name=f"st{qb}_{kb}_{pair}")
            qsl = slice(qb * 512, (qb + 1) * 512)
            ksl = slice(kb * 128, (kb + 1) * 128)
            for j in range(2):
                psl = slice(j * 64, (j + 1) * 64)
                nc.tensor.matmul(
                    st[:, j, :], kT[psl, pair, ksl], qT[psl, pair, qsl],
                    start=True, stop=True)
            n = kb * 2 + pair
            if XDVE_MOD and n % XDVE_MOD == XDVE_MOD - 1:
                ai = apool.tile([128, 2, 512], I16, tag="A",
                                name=f"Ai{qb}_{kb}_{pair}")
                nc.vector.tensor_scalar(
                    ai[:], st[:], FEXP_K, FEXP_B,
                    mybir.AluOpType.mult, mybir.AluOpType.add)
                a_tiles[(kb, pair)] = ai.bitcast(BF16)
            else:
                a = apool.tile([128, 2, 512], BF16, tag="A",
                               name=f"A{qb}_{kb}_{pair}")
                nc.scalar.activation(a[:], st[:], EXP)
                a_tiles[(kb, pair)] = a

        def make_pv_closures(qb, g):
            def run_qq(qq):
                scr = ps.tile([128, 4, HD + 1], F32, tag="aux",
                              name=f"scr{qb}_{g}_{qq}",
                              padded_shape=[128, 4, 128])
                qoff = qq * 128
                for i in range(GRP):
                    kb = g * GRP + i
                    for h in range(4):
                        pair, j = h // 2, h % 2
                        nc.tensor.matmul(
                            scr[:, h, :],
                            a_tiles[(kb, pair)][:, j, qoff:qoff + 128],
                            v_aug[:, kb, h, :],
                            start=(i == 0 and h == 0),
                            stop=(i == GRP - 1 and h == 3))
                qqg = qb * 4 + qq
                if g == 0:
                    nc.vector.tensor_copy(ctxacc[:, qqg, :, :], scr[:])
                else:
                    nc.vector.tensor_add(
                        ctxacc[:, qqg, :, :], ctxacc[:, qqg, :, :], scr[:])
            return [lambda qq=qq: run_qq(qq) for qq in range(4)]

        def finish_pieces(qb):
            """Per-qb epilogue as closures: 1/r scale (DVE), transpose (PE),
            out projection, staging copy, one output DMA."""
            ctxTs = [None] * 4
            ctx2s = [None] * 4
            cell = {}

            def piece_a(qq):
                qqg = qb * 4 + qq
                rinv = mp.tile([128, 4, 1], F32, tag="rinv", name="rinv")
                nc.vector.reciprocal(rinv[:], ctxacc[:, qqg, :, HD:HD + 1])
                ctxT = mp.tile([128, 4, HD], BF16, tag="ctxT", name="ctxT",
                               bufs=3)
                for h in range(4):
                    nc.vector.tensor_scalar_mul(
                        ctxT[:, h, :], ctxacc[:, qqg, h, 0:HD],
                        rinv[:, h, :])
                ctxTs[qq] = ctxT

            def piece_b(qq):
                c2 = mp.tile([128, 2, 128], BF16, tag="ctx2", name="ctx2",
                             bufs=6)
                for j in range(2):
                    tp = ps.tile([128, 128], BF16, tag="aux", name="tp",
                                 padded_shape=[128, 1024])
                    nc.tensor.transpose(
                        tp[:], ctxTs[qq][:, 2 * j:2 * j + 2, :], eye_sb[:])
                    nc.vector.tensor_copy(c2[:, j, :], tp[:])
                ctx2s[qq] = c2

            def piece_c(qq, halfo):
                if "ob" not in cell:
                    cell["ob"] = mp.tile([128, 4, HID], BF16, tag="ob",
                                         name="ob")
                po = ps.tile([128, 512], F32, tag="aux", name="po")
                osl = slice(halfo * 512, (halfo + 1) * 512)
                for j in range(2):
                    nc.tensor.matmul(
                        po[:], ctx2s[qq][:, j, :], wo_sb[:, j, osl],
                        start=(j == 0), stop=(j == 1))
                nc.vector.tensor_copy(cell["ob"][:, qq, osl], po[:])

            def piece_d(qq):
                nc.sync.dma_start(o_part[qb, :, qq, :], cell["ob"][:, qq, :])

            pieces = []
            for qq in range(4):
                pieces.append(lambda qq=qq: piece_a(qq))
                pieces.append(lambda qq=qq: piece_b(qq))
            for qq in range(4):
                for halfo in range(2):
                    pieces.append(
                        lambda qq=qq, h=halfo: piece_c(qq, h))
                pieces.append(lambda qq=qq: piece_d(qq))
            return pieces

        def sweep(qb, interleave):
            for kb in range(NKB):
                # pop every other k-block: 4 closures arrive per GRP=8
                # window, and spacing them keeps the PE's PV bursts small
                # enough for the 3-deep score ring to absorb (the Act
                # engine paces the steady state on HW).
                if pending_pv and kb % 2 == 0:
                    pending_pv.pop(0)()
                for fn in interleave.pop(kb, []):
                    fn()
                emit_scores(qb, kb, 0)   # head pair 0
                emit_scores(qb, kb, 1)   # head pair 1
                if (kb + 1) % GRP == 0:
                    pending_pv.extend(make_pv_closures(qb, kb // GRP))

        # -------- prologue: only what qb0's first k-blocks need --------
        # (scores for q-block qb only read qT rows qb*512..qb*512+511, i.e.
        # q-projection slab qb — slabs 1..3 are deferred into the sweeps)
        # DMAs are emitted in first-use order: the cost model (and HW queue
        # pressure) serializes transfers in program order.
        slabs = {}
        # split wq so the very first projection strip starts ~1.5us earlier
        nc.sync.dma_start(wq_sb[:, 0:NCH // 2, :], wq[:, 0:NCH // 2, :])
        nc.sync.dma_start(wq_sb[:, NCH // 2:, :], wq[:, NCH // 2:, :])
        slabs[("q", 0)] = load_xslab(xqT, 0, split=True)
        nc.sync.dma_start(wk_sb[:], wk[:, :, :])
        slabs[("k", 0)] = load_xslab(xkT, 0, split=True)
        nc.sync.dma_start(wv_sb[:], wv[:, :, :])
        slabs[("v", 0)] = load_xslab(xvT, 0, split=True)
        nc.sync.dma_start(eye_sb[:], eye[:, :])
        nc.sync.dma_start(wo_sb[:], wo[:, :, :])
        for i in range(2):
            proj_strip(slabs[("q", 0)], wq_sb, qT, slice(0, 512), i)
        for i in range(2):
            proj_strip(slabs[("k", 0)], wk_sb, kT, slice(0, 512), i)
        for r in range(4):
            v_group(slabs[("v", 0)], 0, r)

        # qb0: K/V slabs 1..7 spread just-in-time across the k-loop
        inter0 = {}

        def k_strip(s, i):
            proj_strip(slabs[("k", s)], wk_sb, kT,
                       slice(s * 512, (s + 1) * 512), i)

        def q_strip(s, i):
            proj_strip(slabs[("q", s)], wq_sb, qT,
                       slice(s * 512, (s + 1) * 512), i)

        for s in range(1, 8):
            inter0.setdefault(4 * s - 4, []).append(
                lambda s=s: slabs.__setitem__(("k", s), load_xslab(xkT, s)))
            inter0.setdefault(4 * s - 3, []).append(
                lambda s=s: slabs.__setitem__(("v", s), load_xslab(xvT, s)))
            inter0.setdefault(4 * s - 2, []).append(lambda s=s: k_strip(s, 0))
            inter0.setdefault(4 * s - 1, []).append(lambda s=s: k_strip(s, 1))
            for r in range(4):
                inter0.setdefault(4 * s + r, []).append(
                    lambda s=s, r=r: v_group(slabs[("v", s)], s, r))
        # q slab for sweep qb is loaded and projected near the end of sweep
        # qb-1 (load kb25, strips kb28/30; K/V strip emission ends by kb27)
        inter0.setdefault(25, []).append(
            lambda: slabs.__setitem__(("q", 1), load_xslab(xqT, 1)))
        inter0.setdefault(28, []).append(lambda: q_strip(1, 0))
        inter0.setdefault(30, []).append(lambda: q_strip(1, 1))
        sweep(0, inter0)

        for qb in range(1, NQB):
            pieces = finish_pieces(qb - 1)
            inter = {1 + k: [p] for k, p in enumerate(pieces)}
            if qb < NQB - 1:
                nq = qb + 1
                inter.setdefault(25, []).append(
                    lambda s=nq: slabs.__setitem__(
                        ("q", s), load_xslab(xqT, s)))
                inter.setdefault(28, []).append(
                    lambda s=nq: q_strip(s, 0))
                inter.setdefault(30, []).append(
                    lambda s=nq: q_strip(s, 1))
            sweep(qb, inter)

        # drain: interleave the last q-block's finish pieces with the
        # remaining PV pops so the DVE/PE epilogue chains pipeline — each
        # qq's chain (a: scale, b: transpose) starts right after its pop.
        drain = list(pending_pv)
        pending_pv.clear()
        pieces = finish_pieces(NQB - 1)
        order = [drain[0], pieces[0], drain[1], pieces[1], pieces[2],
                 drain[2], pieces[3], pieces[4], drain[3], pieces[5],
                 pieces[6], pieces[7]] + pieces[8:]
        for f in order:
            f()

        _pools.close()

    nc.compile()
    return nc


def _get_nc():
    global _CACHED_NC
    if _CACHED_NC is None:
        _CACHED_NC = _build()
    return _CACHED_NC


def _slabify(xT_bf16):
    """[HID, L] -> [L//512, 128, NCH, 512] (slab-major, contiguous DMA)."""
    L = xT_bf16.shape[1]
    return np.ascontiguousarray(
        xT_bf16.reshape(NCH, 128, L // 512, 512).transpose(2, 1, 0, 3))


def _wslab(w_bf16):
    """[HID, n] -> [128, NCH, n]."""
    n = w_bf16.shape[1]
    return np.ascontiguousarray(
        w_bf16.reshape(NCH, 128, n).transpose(1, 0, 2))


def make_in_maps(query, key, value, w_q, w_k, w_v, w_o):
    import ml_dtypes
    bf = ml_dtypes.bfloat16
    qs = SCALE  # fold the softmax scale into W_q
    xq = [_slabify(query[b].T.astype(bf)) for b in range(B)]
    xk = [_slabify(key[b].T.astype(bf)) for b in range(B)]
    xv = [_slabify(value[b].T.astype(bf)) for b in range(B)]
    eye = np.eye(128, dtype=ml_dtypes.bfloat16)
    ins = []
    for core in range(NCORE):
        b, t = core // 4, core % 4
        csl = slice(t * TC, (t + 1) * TC)
        ins.append({
            "xqT": xq[b],
            "xkT": xk[b],
            "xvT": xv[b],
            "wq": _wslab((w_q[:, csl] * qs).astype(bf)),
            "wk": _wslab(w_k[:, csl].astype(bf)),
            "wv": _wslab(w_v[:, csl].astype(bf)),
            "wo": np.ascontiguousarray(
                w_o[csl, :].astype(bf).reshape(2, 128, HID)
                .transpose(1, 0, 2)),
            "eye": eye,
        })
    return ins


def assemble(results):
    out = np.empty((B, Q, HID), np.float32)
    for b in range(B):
        acc = results[b * 4]["o_part"].astype(np.float32)
        for t in range(1, 4):
            acc += results[b * 4 + t]["o_part"].astype(np.float32)
        # o_part[qb, p, qq, :] -> rows qb*512 + qq*128 + p
        out[b] = acc.transpose(0, 2, 1, 3).reshape(Q, HID)
    return out


_EXEC = None


def _get_exec():
    """Build the 8-core shard_map executable once; reuse across calls."""
    global _EXEC
    if _EXEC is not None:
        return _EXEC
    import jax
    from jax.sharding import Mesh, PartitionSpec
    from jax.experimental.shard_map import shard_map
    from concourse.bass2jax import (_bass_exec_p, install_neuronx_cc_hook,
                                    partition_id_tensor)

    install_neuronx_cc_hook()
    nc = _get_nc()
    in_names, out_names, out_avals, zero_outs = [], [], [], []
    for alloc in nc.m.functions[0].allocations:
        if not isinstance(alloc, mybir.MemoryLocationSet):
            continue
        name = alloc.memorylocations[0].name
        if alloc.kind == "ExternalInput":
            if name != "partition_id":
                in_names.append(name)
        elif alloc.kind == "ExternalOutput":
            out_names.append(name)
            shape = tuple(alloc.tensor_shape)
            dtype = mybir.dt.np(alloc.dtype)
            out_avals.append(jax.core.ShapedArray(shape, dtype))
            zero_outs.append(np.zeros(shape, dtype))
    partition_name = (nc.partition_id_tensor.name
                      if nc.partition_id_tensor else None)
    all_in = list(in_names) + list(out_names)
    if partition_name:
        all_in.append(partition_name)

    def _body(*args):
        operands = list(args)
        if partition_name is not None:
            operands.append(partition_id_tensor())
        return tuple(_bass_exec_p.bind(
            *operands, out_avals=tuple(out_avals), in_names=tuple(all_in),
            out_names=tuple(out_names), lowering_input_output_aliases=(),
            sim_require_finite=True, sim_require_nnan=True, nc=nc))

    devices = jax.devices()[:NCORE]
    mesh = Mesh(np.asarray(devices), ("core",))
    n_all = len(in_names) + len(out_names)
    fn = jax.jit(shard_map(_body, mesh=mesh,
                           in_specs=(PartitionSpec("core"),) * n_all,
                           out_specs=(PartitionSpec("core"),) * len(out_names),
                           check_rep=False), keep_unused=True)
    concat_zeros = [np.zeros((NCORE * z.shape[0], *z.shape[1:]), z.dtype)
                    for z in zero_outs]
    _EXEC = (fn, in_names, out_names, out_avals, concat_zeros)
    return _EXEC


def kernel(query, key, value, w_q, w_k, w_v, w_o):
    query = np.asarray(query, dtype=np.float32)
    key = np.asarray(key, dtype=np.float32)
    value = np.asarray(value, dtype=np.float32)
    ins = make_in_maps(query, key, value, np.asarray(w_q, np.float32),
                       np.asarray(w_k, np.float32),
                       np.asarray(w_v, np.float32),
                       np.asarray(w_o, np.float32))
    fn, in_names, out_names, out_avals, concat_zeros = _get_exec()
    concat_in = [np.concatenate([np.asarray(ins[c][nm]) for c in range(NCORE)])
                 for nm in in_names]
    out_arrs = fn(*concat_in, *concat_zeros)
    results = [
        {nm: np.asarray(out_arrs[i]).reshape(NCORE, *out_avals[i].shape)[c]
         for i, nm in enumerate(out_names)}
        for c in range(NCORE)]
    return assemble(results)


if __name__ == "__main__":
    np.random.seed(0)
    q = np.random.randn(B, Q, HID).astype(np.float32)
    k = np.random.randn(B, KL, HID).astype(np.float32)
    v = np.random.randn(B, KL, HID).astype(np.float32)
    s = 1.0 / np.sqrt(HID)
    wq_ = (np.random.randn(HID, HID) * s).astype(np.float32)
    wk_ = (np.random.randn(HID, HID) * s).astype(np.float32)
    wv_ = (np.random.randn(HID, HID) * s).astype(np.float32)
    wo_ = (np.random.randn(HID, HID) * s).astype(np.float32)
    t0 = time.time()
    out = kernel(q, k, v, wq_, wk_, wv_, wo_)
    print("kernel done", time.time() - t0, out.shape)
